# revision 19
# baseline (speedup 1.0000x reference)
"""HODLR matvec kernel for 8 TRN2 NeuronCores (Bass/Tile), v4.

Sharding: node axis split into 8 contiguous slices of 32768 nodes.
DMA-bound design (~38MB/core at ~360GB/s): all compute hides under the
input stream.

Per core:
  stream order: xt | UPA (u levels 0-2, n-major) | UPB (levels 3-7) |
    UT23 (u^T levels 4-7, fully prefetched) | UT01 (levels 0-3, ring),
    with corr output DMAs at the queue tails.
  projection  one DR fp8 matmul per chunk-pair per phase with x as the
    stationary. UPA accumulates [64,192] (levels 0-2) for the whole
    core, closing as soon as UPA lands -> early AllGather. UPB
    accumulates [64,320] windows of 4 chunk-pairs (= level-7 blocks,
    ping-ponging 2 PSUM banks); coarser level block sums come from a
    cheap SBUF pairwise tree on gpsimd.
  collective  AllGather of the [64, 192] level-0..2 partials; masked
    receive-combine (sender-invariant).
  transposes  PE is_transpose matmuls turn sibling-selected t^T slices
    of the window/tree buffers into fp8 DR stationaries (sibling XOR
    encoded in the source APs).
  expansion   per 512-node group: one DR matmul vs UT23 (levels 4-7,
    PSUM start) and one vs UT01 (levels 0-3, stop); drains write fp8
    corr staged per 4096 block (corr is ~1e-2 of y, so fp8 keeps the
    overall error ~1e-4).
u/x fed as fp8e4m3 (u scaled by USCALE; host divides the correction by
USCALE^2 and adds diag*x in fp32).
"""

import os
import sys

sys.path.insert(0, "/opt/trn_rl_repo")

import numpy as np
import ml_dtypes

BF16 = ml_dtypes.bfloat16
FP8 = ml_dtypes.float8_e4m3

B = 64
N = 262144
NCORES = 8
M = N // NCORES          # 32768 nodes per core
R = 64
DEPTH = 8
CH = M // 128            # 256 chunks of 128 nodes
CP = CH // 2             # 128 chunk-pairs (256 nodes, DR k-tiles)
USCALE = 64.0

_cached = {}


def _build_bass():
    import concourse.bacc as bacc
    import concourse.tile as tile
    import concourse.mybir as mybir
    from contextlib import ExitStack

    BF = mybir.dt.bfloat16
    F8 = mybir.dt.float8e4
    F32 = mybir.dt.float32
    ADD = mybir.AluOpType.add
    MULT = mybir.AluOpType.mult
    DR = mybir.MatmulPerfMode.DoubleRow

    nc = bacc.Bacc(
        "TRN2",
        target_bir_lowering=False,
        debug=False,
        enable_asserts=False,
        num_devices=NCORES,
    )

    xt_d = nc.dram_tensor("xt", [128, CH, B], F8, kind="ExternalInput").ap()
    upa_d = nc.dram_tensor("upa", [128, CP, 2, 192], F8, kind="ExternalInput").ap()
    upb_d = nc.dram_tensor("upb", [128, CP, 2, 320], F8, kind="ExternalInput").ap()
    ut_d = nc.dram_tensor("ut", [4, 128, M], F8, kind="ExternalInput").ap()
    msk_d = nc.dram_tensor("mask", [64, 8, 192], BF, kind="ExternalInput").ap()
    idn_d = nc.dram_tensor("idn", [64, 64], BF, kind="ExternalInput").ap()
    corr_d = nc.dram_tensor("corr", [B, M], F8, kind="ExternalOutput").ap()

    NSL = 8            # slices for UPA/UPB streams
    SLC = CP // NSL    # 16 chunk-pairs per slice

    with tile.TileContext(nc) as tc, ExitStack() as ctx:
        const = ctx.enter_context(tc.tile_pool(name="const", bufs=1))
        upap = ctx.enter_context(tc.tile_pool(name="upap", bufs=2))
        upbp = ctx.enter_context(tc.tile_pool(name="upbp", bufs=2))
        utap = ctx.enter_context(tc.tile_pool(name="utap", bufs=4))
        utbp = ctx.enter_context(tc.tile_pool(name="utbp", bufs=5))
        y23p = ctx.enter_context(tc.tile_pool(name="y23p", bufs=8))
        ttp = ctx.enter_context(tc.tile_pool(name="ttp", bufs=1))
        statp = ctx.enter_context(tc.tile_pool(name="statp", bufs=1))
        yop = ctx.enter_context(tc.tile_pool(name="yop", bufs=2))
        # PSUM (8 banks): pW bufs=2 (UPB windows ping-pong, then the
        # transposes), pF + pe1..pe5 bufs=1 (psF / 6-deep eps ring)
        pW = ctx.enter_context(tc.tile_pool(name="pW", bufs=2, space="PSUM"))
        pF = ctx.enter_context(tc.tile_pool(name="pF", bufs=1, space="PSUM"))
        pes = [
            ctx.enter_context(
                tc.tile_pool(name=f"pe{i}", bufs=1, space="PSUM")
            )
            for i in range(5)
        ]
        dram = ctx.enter_context(tc.tile_pool(name="dram", bufs=1, space="DRAM"))

        def drainer():
            i = 0
            while True:
                yield (nc.vector if i % 2 == 0 else nc.any)
                i += 1

        dr_eng = drainer()

        # ---------------- constants ----------------
        xt = const.tile([128, CH, B], F8, tag="xt")
        nc.sync.dma_start(xt[:, 0 : CH // 2, :], xt_d[:, 0 : CH // 2, :])
        nc.scalar.dma_start(xt[:, CH // 2 :, :], xt_d[:, CH // 2 :, :])
        msk = const.tile([64, 8, 192], BF, tag="msk")
        nc.scalar.dma_start(msk[:], msk_d[:])
        idn = const.tile([64, 64], BF, tag="idn")
        nc.scalar.dma_start(idn[:], idn_d[:])

        # ---------------- projection: levels 0-2 (UPA) ----------------
        psF_t = pF.tile([64, 512], F32, tag="t", name="psF")
        psF = psF_t[:, 0:192]
        for s in range(NSL):
            ua = upap.tile([128, SLC, 2, 192], F8, tag="upa", name=f"upa{s}")
            (nc.sync if s % 2 == 0 else nc.scalar).dma_start(
                ua[:], upa_d[:, SLC * s : SLC * (s + 1), :, :]
            )
            for i in range(SLC):
                cp = SLC * s + i
                nc.tensor.matmul(
                    psF,
                    xt[:, 2 * cp : 2 * cp + 2, :],
                    ua[:, i, :, :],
                    start=(cp == 0),
                    stop=(cp == CP - 1),
                    perf_mode=DR,
                )

        # collective staging + AllGather (launches ~25% into the stream)
        b_in = dram.tile([64, 192], BF, tag="b_in")
        b_out = dram.tile([8, 64, 192], BF, tag="b_out", addr_space="Shared")
        sb_in = statp.tile([64, 192], BF, tag="sb_in")
        nc.vector.tensor_copy(sb_in[:], psF)
        nc.gpsimd.dma_start(b_in[:], sb_in[:])
        nc.gpsimd.collective_compute(
            "AllGather",
            mybir.AluOpType.bypass,
            replica_groups=[list(range(NCORES))],
            ins=[b_in.opt()],
            outs=[b_out.opt()],
        )
        recv = statp.tile([64, 8, 192], BF, tag="recv")
        nc.gpsimd.dma_start(recv[:], b_out.rearrange("k b r -> b k r"))
        TT = ttp.tile([64, 3, 64], BF, tag="TT")

        # ---------------- projection: levels 3-7 (UPB) ----------------
        # one matmul per chunk-pair into a [64, 320] window accumulator
        # (4 cps = one level-7 block), ping-ponging two banks; block
        # sums for levels 3-6 via SBUF pairwise tree on gpsimd.
        Wb = ttp.tile([64, 32, 320], BF, tag="Wb")
        S1 = ttp.tile([64, 16, 256], BF, tag="S1")
        S2 = ttp.tile([64, 8, 192], BF, tag="S2")
        S3 = ttp.tile([64, 4, 128], BF, tag="S3")
        S4 = ttp.tile([64, 2, 64], BF, tag="S4")
        cur = [None]
        for s in range(NSL):
            ub = upbp.tile([128, SLC, 2, 320], F8, tag="upb", name=f"upb{s}")
            (nc.sync if s % 2 == 0 else nc.scalar).dma_start(
                ub[:], upb_d[:, SLC * s : SLC * (s + 1), :, :]
            )
            for i in range(SLC):
                cp = SLC * s + i
                if cp % 4 == 0:
                    cur[0] = pW.tile(
                        [64, 512], F32, tag="t", name=f"W{cp // 4}"
                    )
                nc.tensor.matmul(
                    cur[0][:, 0:320],
                    xt[:, 2 * cp : 2 * cp + 2, :],
                    ub[:, i, :, :],
                    start=(cp % 4 == 0),
                    stop=(cp % 4 == 3),
                    perf_mode=DR,
                )
                if cp % 4 == 3:
                    next(dr_eng).tensor_copy(
                        Wb[:, cp // 4, :], cur[0][:, 0:320]
                    )
                # incremental pairwise tree: each sum emitted as soon
                # as both inputs exist, pipelined under the UPB stream
                if cp % 8 == 7:
                    j = cp // 8
                    next(dr_eng).tensor_tensor(
                        S1[:, j, :], Wb[:, 2 * j, 0:256],
                        Wb[:, 2 * j + 1, 0:256], op=ADD,
                    )
                if cp % 16 == 15:
                    j = cp // 16
                    next(dr_eng).tensor_tensor(
                        S2[:, j, :], S1[:, 2 * j, 0:192],
                        S1[:, 2 * j + 1, 0:192], op=ADD,
                    )
                if cp % 32 == 31:
                    j = cp // 32
                    next(dr_eng).tensor_tensor(
                        S3[:, j, :], S2[:, 2 * j, 0:128],
                        S2[:, 2 * j + 1, 0:128], op=ADD,
                    )
                if cp % 64 == 63:
                    j = cp // 64
                    next(dr_eng).tensor_tensor(
                        S4[:, j, :], S3[:, 2 * j, 0:64],
                        S3[:, 2 * j + 1, 0:64], op=ADD,
                    )


        # t^T source slices per (level, block)
        def tsrc(l, blk):
            if l == 7:
                return Wb[:, blk, 256:320]
            if l == 6:
                return S1[:, blk, 192:256]
            if l == 5:
                return S2[:, blk, 128:192]
            if l == 4:
                return S3[:, blk, 64:128]
            if l == 3:
                return S4[:, blk, 0:64]
            return TT[:, l, :]

        # ---------------- sibling transposes -> fp8 stationaries -------
        # P23[:, m7, 0, :] = (t4sib | t5sib); [:, m7, 1, :] = (t6sib | t7sib)
        # P01[:, m3, 0, :] = (L0comb | L1comb); [:, m3, 1, :] = (L2comb | t3sib)
        P23 = statp.tile([128, 32, 2, B], F8, tag="P23")
        P01 = statp.tile([128, 2, 2, B], F8, tag="P01")

        def transpose_pair(dst_ap, src_top, src_bot, nm):
            # full-bank scratch keeps start-flag zeroing away from
            # neighbors
            tp = pW.tile([128, 1024], BF, tag="t", name=nm)
            nc.tensor.matmul(
                tp[0:64, 0:B], src_top, idn[:], is_transpose=True
            )
            nc.tensor.matmul(
                tp[64:128, 0:B], src_bot, idn[:], is_transpose=True
            )
            next(dr_eng).tensor_copy(dst_ap, tp[:, 0:B])

        for m7 in range(32):
            transpose_pair(
                P23[:, m7, 0, :],
                tsrc(4, (m7 // 8) ^ 1),
                tsrc(5, (m7 // 4) ^ 1),
                f"tp45_{m7}",
            )
            transpose_pair(
                P23[:, m7, 1, :],
                tsrc(6, (m7 // 2) ^ 1),
                tsrc(7, m7 ^ 1),
                f"tp67_{m7}",
            )
        # ---------------- UT prefetch ----------------
        # UT23 fully resident (no backpressure); UT01 4-deep ring
        GBORD = list(range(8))
        u23s = {}
        for gb in GBORD:
            u23 = utap.tile([128, 2, 4096], F8, tag="ut23", name=f"u23_{gb}")
            for f in range(2):
                nc.sync.dma_start(
                    u23[:, f, :], ut_d[2 + f, :, 4096 * gb : 4096 * (gb + 1)]
                )
            u23s[gb] = u23

        # -------- expansion pass A: levels 4-7 (no collective dep) -----
        # fp8 partials land in slots recycled from the retiring u23
        # prefetch buffers (same pool tag -> same SBUF ring).
        # 8-deep eps ring (pW's two banks are free once the P23
        # transposes retire): no matmul of a block waits on its own
        # block's drains, which would cycle with the y23 slot reuse.
        erng = [pF] + pes + [pW, pW]
        y23s = {}
        for gi, gb in enumerate(GBORD):
            y23 = y23p.tile([B, 4096], F8, tag="y23", name=f"y23_{gb}")
            y23s[gb] = y23
            for gg in range(8):
                g = 8 * gb + gg
                eps_t = erng[(8 * gi + gg) % 8].tile(
                    [64, 512], F32, tag="t", name=f"epsA{g}"
                )
                eps = eps_t[:, 0:512]
                sl = slice(512 * gg, 512 * (gg + 1))
                nc.tensor.matmul(
                    eps, P23[:, g // 2, :, :], u23s[gb][:, :, sl],
                    start=True, stop=True, perf_mode=DR,
                )
                next(dr_eng).tensor_copy(y23[:, sl], eps)

        # masked receive-combine -> TT (levels 0,1,2 combined t^T).
        # Emitted after pass A so only pass B waits on the collective.
        for k in range(8):
            nc.vector.tensor_tensor(
                recv[:, k, :], recv[:, k, :], msk[:, k, :], op=MULT
            )
        acc01 = statp.tile([64, 192], BF, tag="acc01")
        nc.vector.tensor_tensor(acc01[:], recv[:, 0, :], recv[:, 1, :], op=ADD)
        for k in range(2, 7):
            nc.vector.tensor_tensor(acc01[:], acc01[:], recv[:, k, :], op=ADD)
        nc.vector.tensor_tensor(TT[:], acc01[:], recv[:, 7, :], op=ADD)
        for m3 in range(2):
            transpose_pair(P01[:, m3, 0, :], tsrc(0, 0), tsrc(1, 0),
                           f"tp01_{m3}")
            transpose_pair(P01[:, m3, 1, :], tsrc(2, 0), tsrc(3, m3 ^ 1),
                           f"tp23_{m3}")

        # -------- expansion pass B: levels 0-3 + accumulate ------------
        for gi, gb in enumerate(GBORD):
            u01 = utbp.tile([128, 2, 4096], F8, tag="ut01", name=f"u01_{gb}")
            for f in range(2):
                (nc.sync if gi % 2 == 0 else nc.scalar).dma_start(
                    u01[:, f, :], ut_d[f, :, 4096 * gb : 4096 * (gb + 1)]
                )
            yo = yop.tile([B, 4096], F8, tag="yo", name=f"yo{gb}")
            for gg in range(8):
                g = 8 * gb + gg
                eps_t = erng[(8 * gi + gg) % 8].tile(
                    [64, 512], F32, tag="t", name=f"epsB{g}"
                )
                eps = eps_t[:, 0:512]
                sl = slice(512 * gg, 512 * (gg + 1))
                nc.tensor.matmul(
                    eps, P01[:, g // 32, :, :], u01[:, :, sl],
                    start=True, stop=True, perf_mode=DR,
                )
                next(dr_eng).tensor_tensor(
                    yo[:, sl], eps, y23s[gb][:, sl], op=ADD
                )
            (nc.sync if gi % 2 == 0 else nc.scalar).dma_start(
                corr_d[:, 4096 * gb : 4096 * (gb + 1)], yo[:]
            )

    nc.compile()
    return nc


def _pack_inputs(x, diag, u):
    """Build per-core input maps. x (B,N,1) f32, u (DEPTH,N,R) f32."""
    in_maps = []
    x2 = np.asarray(x).reshape(B, N)
    u3 = np.asarray(u)
    idn = np.eye(64, dtype=BF16)
    for c in range(NCORES):
        base = c * M
        xsl = x2[:, base : base + M]                      # (B, M)
        us = (u3[:, base : base + M, :] * USCALE).astype(np.float32)
        xt = np.ascontiguousarray(
            xsl.T.reshape(CH, 128, B).transpose(1, 0, 2)
        ).astype(FP8)                                     # [128, CH, B]
        unm = us.transpose(1, 0, 2).reshape(M, 512)       # [n, l*64+r]
        up4 = unm.reshape(CP, 2, 128, 512).transpose(2, 0, 1, 3)
        upa = np.ascontiguousarray(up4[..., 0:192]).astype(FP8)
        upb = np.ascontiguousarray(up4[..., 192:512]).astype(FP8)
        ut = np.ascontiguousarray(
            us.transpose(0, 2, 1).reshape(512, M).reshape(4, 128, M)
        ).astype(FP8)
        msk = np.zeros((64, 8, 192), dtype=BF16)
        for k in range(8):
            if (k // 4) == ((c // 4) ^ 1):
                msk[:, k, 0:64] = 1.0     # level 0
            if (k // 2) == ((c // 2) ^ 1):
                msk[:, k, 64:128] = 1.0   # level 1
            if k == c ^ 1:
                msk[:, k, 128:192] = 1.0  # level 2
        in_maps.append(
            {"xt": xt, "upa": upa, "upb": upb, "ut": ut, "mask": msk,
             "idn": idn}
        )
    return in_maps


last_results = None


def kernel(x, diag, u):
    global last_results
    from concourse.bass_utils import run_bass_kernel_spmd

    if "nc" not in _cached:
        _cached["nc"] = _build_bass()
    nc = _cached["nc"]

    in_maps = _pack_inputs(x, diag, u)
    res = run_bass_kernel_spmd(nc, in_maps, core_ids=list(range(NCORES)))
    last_results = res

    x2 = np.asarray(x, dtype=np.float32).reshape(B, N)
    d2 = np.asarray(diag, dtype=np.float32).reshape(1, N)
    y = d2 * x2
    inv = 1.0 / (USCALE * USCALE)
    for c in range(NCORES):
        y[:, c * M : (c + 1) * M] += res.results[c]["corr"].astype(np.float32) * inv
    return y.reshape(B, N, 1).astype(np.float32)


# revision 24
# speedup vs baseline: 1.0928x; 1.0928x over previous
"""HODLR matvec kernel for 8 TRN2 NeuronCores (Bass/Tile), v4.

Sharding: node axis split into 8 contiguous slices of 32768 nodes.
DMA-bound design (~38MB/core at ~360GB/s): all compute hides under the
input stream.

Per core:
  stream order: xt | UPA (u levels 0-2, n-major) | UPB (levels 3-7) |
    UT23 (u^T levels 4-7, fully prefetched) | UT01 (levels 0-3, ring),
    with corr output DMAs at the queue tails.
  projection  one DR fp8 matmul per chunk-pair per phase with x as the
    stationary. UPA accumulates [64,192] (levels 0-2) for the whole
    core, closing as soon as UPA lands -> early AllGather. UPB
    accumulates [64,320] windows of 4 chunk-pairs (= level-7 blocks,
    ping-ponging 2 PSUM banks); coarser level block sums come from a
    cheap SBUF pairwise tree on gpsimd.
  collective  AllGather of the [64, 192] level-0..2 partials; masked
    receive-combine (sender-invariant).
  transposes  PE is_transpose matmuls turn sibling-selected t^T slices
    of the window/tree buffers into fp8 DR stationaries (sibling XOR
    encoded in the source APs).
  expansion   per 512-node group: one DR matmul vs UT23 (levels 4-7,
    PSUM start) and one vs UT01 (levels 0-3, stop); drains write fp8
    corr staged per 4096 block (corr is ~1e-2 of y, so fp8 keeps the
    overall error ~1e-4).
u/x fed as fp8e4m3 (u scaled by USCALE; host divides the correction by
USCALE^2 and adds diag*x in fp32).
"""

import os
import sys

sys.path.insert(0, "/opt/trn_rl_repo")

import numpy as np
import ml_dtypes

BF16 = ml_dtypes.bfloat16
FP8 = ml_dtypes.float8_e4m3

B = 64
N = 262144
NCORES = 8
M = N // NCORES          # 32768 nodes per core
R = 64
DEPTH = 8
CH = M // 128            # 256 chunks of 128 nodes
CP = CH // 2             # 128 chunk-pairs (256 nodes, DR k-tiles)
USCALE = 64.0

_cached = {}


def _build_bass():
    import concourse.bacc as bacc
    import concourse.tile as tile
    import concourse.mybir as mybir
    from contextlib import ExitStack

    BF = mybir.dt.bfloat16
    F8 = mybir.dt.float8e4
    F32 = mybir.dt.float32
    ADD = mybir.AluOpType.add
    MULT = mybir.AluOpType.mult
    DR = mybir.MatmulPerfMode.DoubleRow

    nc = bacc.Bacc(
        "TRN2",
        target_bir_lowering=False,
        debug=False,
        enable_asserts=False,
        num_devices=NCORES,
    )

    xt_d = nc.dram_tensor("xt", [128, CH, B], F8, kind="ExternalInput").ap()
    upa_d = nc.dram_tensor("upa", [128, CP, 2, 192], F8, kind="ExternalInput").ap()
    upb_d = nc.dram_tensor("upb", [128, CP, 2, 320], F8, kind="ExternalInput").ap()
    ut_d = nc.dram_tensor("ut", [4, 128, M], F8, kind="ExternalInput").ap()
    msk_d = nc.dram_tensor("mask", [64, 8, 192], BF, kind="ExternalInput").ap()
    idn_d = nc.dram_tensor("idn", [64, 64], BF, kind="ExternalInput").ap()
    corr_d = nc.dram_tensor("corr", [B, M], F8, kind="ExternalOutput").ap()

    NSL = 8            # slices for UPA/UPB streams
    SLC = CP // NSL    # 16 chunk-pairs per slice

    with tile.TileContext(nc) as tc, ExitStack() as ctx:
        const = ctx.enter_context(tc.tile_pool(name="const", bufs=1))
        upap = ctx.enter_context(tc.tile_pool(name="upap", bufs=2))
        upbp = ctx.enter_context(tc.tile_pool(name="upbp", bufs=2))
        utap = ctx.enter_context(tc.tile_pool(name="utap", bufs=4))
        utbp = ctx.enter_context(tc.tile_pool(name="utbp", bufs=5))
        y23p = ctx.enter_context(tc.tile_pool(name="y23p", bufs=8))
        ttp = ctx.enter_context(tc.tile_pool(name="ttp", bufs=1))
        statp = ctx.enter_context(tc.tile_pool(name="statp", bufs=1))
        yop = ctx.enter_context(tc.tile_pool(name="yop", bufs=2))
        # PSUM (8 banks): pW bufs=2 (UPB windows ping-pong, then the
        # transposes), pF + pe1..pe5 bufs=1 (psF / 6-deep eps ring)
        pW = ctx.enter_context(tc.tile_pool(name="pW", bufs=2, space="PSUM"))
        pF = ctx.enter_context(tc.tile_pool(name="pF", bufs=1, space="PSUM"))
        pes = [
            ctx.enter_context(
                tc.tile_pool(name=f"pe{i}", bufs=1, space="PSUM")
            )
            for i in range(5)
        ]
        dram = ctx.enter_context(tc.tile_pool(name="dram", bufs=1, space="DRAM"))

        def drainer():
            i = 0
            while True:
                yield (nc.vector if i % 2 == 0 else nc.any)
                i += 1

        dr_eng = drainer()

        # ---------------- constants ----------------
        xt = const.tile([128, CH, B], F8, tag="xt")
        nc.sync.dma_start(xt[:, 0 : CH // 2, :], xt_d[:, 0 : CH // 2, :])
        nc.scalar.dma_start(xt[:, CH // 2 :, :], xt_d[:, CH // 2 :, :])
        msk = const.tile([64, 8, 192], BF, tag="msk")
        nc.scalar.dma_start(msk[:], msk_d[:])
        idn = const.tile([64, 64], BF, tag="idn")
        nc.scalar.dma_start(idn[:], idn_d[:])

        # ---------------- projection: levels 0-2 (UPA) ----------------
        psF_t = pF.tile([64, 512], F32, tag="t", name="psF")
        psF = psF_t[:, 0:192]
        for s in range(NSL):
            ua = upap.tile([128, SLC, 2, 192], F8, tag="upa", name=f"upa{s}")
            (nc.sync if s % 2 == 0 else nc.scalar).dma_start(
                ua[:], upa_d[:, SLC * s : SLC * (s + 1), :, :]
            )
            for i in range(SLC):
                cp = SLC * s + i
                nc.tensor.matmul(
                    psF,
                    xt[:, 2 * cp : 2 * cp + 2, :],
                    ua[:, i, :, :],
                    start=(cp == 0),
                    stop=(cp == CP - 1),
                    perf_mode=DR,
                )

        class _TpRing:
            def __init__(self, pools):
                self.pools = pools
                self.i = 0

            def tile(self, *a, **kw):
                p = self.pools[self.i % len(self.pools)]
                self.i += 1
                return p.tile(*a, **kw)

        tp_ring = [None]

        # collective staging + AllGather (launches ~25% into the stream)
        b_in = dram.tile([64, 192], BF, tag="b_in")
        b_out = dram.tile([8, 64, 192], BF, tag="b_out", addr_space="Shared")
        sb_in = statp.tile([64, 192], BF, tag="sb_in")
        nc.vector.tensor_copy(sb_in[:], psF)
        tp_ring[0] = _TpRing([pW, pF, pW])
        nc.gpsimd.dma_start(b_in[:], sb_in[:])
        nc.gpsimd.collective_compute(
            "AllGather",
            mybir.AluOpType.bypass,
            replica_groups=[list(range(NCORES))],
            ins=[b_in.opt()],
            outs=[b_out.opt()],
        )
        recv = statp.tile([64, 8, 192], BF, tag="recv")
        nc.gpsimd.dma_start(recv[:], b_out.rearrange("k b r -> b k r"))
        TT = ttp.tile([64, 3, 64], BF, tag="TT")

        # sibling transposes -> fp8 DR stationaries.
        # P23[:, m7, 0, :] = (t4sib | t5sib); [:, m7, 1, :] = (t6sib | t7sib)
        # P01[:, m3, 0, :] = (L0comb | L1comb); [:, m3, 1, :] = (L2comb | t3sib)
        # m7 0-15 only need windows <= 15 via the sibling XOR, so they are
        # emitted mid-UPB; 16-31 need window 31 and go after the loop.
        P23 = statp.tile([128, 32, 2, B], F8, tag="P23")
        P01 = statp.tile([128, 2, 2, B], F8, tag="P01")


        def transpose_quad(dst_ap, srcs, nm):
            # 4 transposes batched into one full-bank psum tile + 1 drain
            tp = tp_ring[0].tile([128, 1024], BF, tag="t", name=nm)
            for q, srcq in enumerate(srcs):
                # q0/q1 carry start=True (zero-marks the bank per
                # partition half); q2/q3 reuse those marks lazily so no
                # second zeroing clips the already-written columns
                nc.tensor.matmul(
                    tp[64 * (q % 2) : 64 * (q % 2) + 64,
                       64 * (q // 2) : 64 * (q // 2) + B],
                    srcq, idn[:], is_transpose=True,
                    start=(q < 2), stop=True,
                )
            next(dr_eng).tensor_copy(dst_ap, tp[:, 0:128])

        def emit_tp(m7):
            transpose_quad(
                P23[:, m7, :, :],
                (tsrc(4, (m7 // 8) ^ 1), tsrc(5, (m7 // 4) ^ 1),
                 tsrc(6, (m7 // 2) ^ 1), tsrc(7, m7 ^ 1)),
                f"tpq_{m7}",
            )

        # ---------------- projection: levels 3-7 (UPB) ----------------
        # one matmul per chunk-pair into a [64, 320] window accumulator
        # (4 cps = one level-7 block), ping-ponging two banks; block
        # sums for levels 3-6 via SBUF pairwise tree on gpsimd.
        Wb = ttp.tile([64, 32, 320], BF, tag="Wb")
        S1 = ttp.tile([64, 16, 256], BF, tag="S1")
        S2 = ttp.tile([64, 8, 192], BF, tag="S2")
        S3 = ttp.tile([64, 4, 128], BF, tag="S3")
        S4 = ttp.tile([64, 2, 64], BF, tag="S4")
        def tsrc(l, blk):
            if l == 7:
                return Wb[:, blk, 256:320]
            if l == 6:
                return S1[:, blk, 192:256]
            if l == 5:
                return S2[:, blk, 128:192]
            if l == 4:
                return S3[:, blk, 64:128]
            if l == 3:
                return S4[:, blk, 0:64]
            return TT[:, l, :]

        cur = [None]
        for s in range(NSL):
            ub = upbp.tile([128, SLC, 2, 320], F8, tag="upb", name=f"upb{s}")
            (nc.sync if s % 2 == 0 else nc.scalar).dma_start(
                ub[:], upb_d[:, SLC * s : SLC * (s + 1), :, :]
            )
            for i in range(SLC):
                cp = SLC * s + i
                if cp % 4 == 0:
                    cur[0] = pW.tile(
                        [64, 512], F32, tag="t", name=f"W{cp // 4}"
                    )
                nc.tensor.matmul(
                    cur[0][:, 0:320],
                    xt[:, 2 * cp : 2 * cp + 2, :],
                    ub[:, i, :, :],
                    start=(cp % 4 == 0),
                    stop=(cp % 4 == 3),
                    perf_mode=DR,
                )
                if cp % 4 == 3:
                    next(dr_eng).tensor_copy(
                        Wb[:, cp // 4, :], cur[0][:, 0:320]
                    )
                # incremental pairwise tree: each sum emitted as soon
                # as both inputs exist, pipelined under the UPB stream
                if cp % 8 == 7:
                    j = cp // 8
                    next(dr_eng).tensor_tensor(
                        S1[:, j, :], Wb[:, 2 * j, 0:256],
                        Wb[:, 2 * j + 1, 0:256], op=ADD,
                    )
                if cp % 16 == 15:
                    j = cp // 16
                    next(dr_eng).tensor_tensor(
                        S2[:, j, :], S1[:, 2 * j, 0:192],
                        S1[:, 2 * j + 1, 0:192], op=ADD,
                    )
                if cp % 32 == 31:
                    j = cp // 32
                    next(dr_eng).tensor_tensor(
                        S3[:, j, :], S2[:, 2 * j, 0:128],
                        S2[:, 2 * j + 1, 0:128], op=ADD,
                    )
                if cp % 64 == 63:
                    j = cp // 64
                    next(dr_eng).tensor_tensor(
                        S4[:, j, :], S3[:, 2 * j, 0:64],
                        S3[:, 2 * j + 1, 0:64], op=ADD,
                    )
                if cp == 63:
                    for m7 in range(16):
                        emit_tp(m7)


        for m7 in range(16, 32):
            emit_tp(m7)

        # ---------------- UT prefetch ----------------
        # UT23 fully resident (no backpressure); UT01 4-deep ring
        GBORD = list(range(8))
        u23s = {}
        for gb in GBORD:
            u23 = utap.tile([128, 2, 4096], F8, tag="ut23", name=f"u23_{gb}")
            for f in range(2):
                nc.sync.dma_start(
                    u23[:, f, :], ut_d[2 + f, :, 4096 * gb : 4096 * (gb + 1)]
                )
            u23s[gb] = u23

        # -------- expansion pass A: levels 4-7 (no collective dep) -----
        # fp8 partials land in slots recycled from the retiring u23
        # prefetch buffers (same pool tag -> same SBUF ring).
        # 8-deep eps ring (pW's two banks are free once the P23
        # transposes retire): no matmul of a block waits on its own
        # block's drains, which would cycle with the y23 slot reuse.
        erng = pes + [pW, pW]
        y23s = {}
        for gi, gb in enumerate(GBORD):
            y23 = y23p.tile([B, 4096], F8, tag="y23", name=f"y23_{gb}")
            y23s[gb] = y23
            for gg in range(8):
                g = 8 * gb + gg
                eps_t = erng[(8 * gi + gg) % 7].tile(
                    [64, 512], F32, tag="t", name=f"epsA{g}"
                )
                eps = eps_t[:, 0:512]
                sl = slice(512 * gg, 512 * (gg + 1))
                nc.tensor.matmul(
                    eps, P23[:, g // 2, :, :], u23s[gb][:, :, sl],
                    start=True, stop=True, perf_mode=DR,
                )
                next(dr_eng).tensor_copy(y23[:, sl], eps)

        # masked receive-combine -> TT (levels 0,1,2 combined t^T).
        # Emitted after pass A so only pass B waits on the collective.
        for k in range(8):
            nc.vector.tensor_tensor(
                recv[:, k, :], recv[:, k, :], msk[:, k, :], op=MULT
            )
        acc01 = statp.tile([64, 192], BF, tag="acc01")
        nc.vector.tensor_tensor(acc01[:], recv[:, 0, :], recv[:, 1, :], op=ADD)
        for k in range(2, 7):
            nc.vector.tensor_tensor(acc01[:], acc01[:], recv[:, k, :], op=ADD)
        nc.vector.tensor_tensor(TT[:], acc01[:], recv[:, 7, :], op=ADD)
        for m3 in range(2):
            transpose_quad(
                P01[:, m3, :, :],
                (tsrc(0, 0), tsrc(1, 0), tsrc(2, 0), tsrc(3, m3 ^ 1)),
                f"tpq01_{m3}",
            )

        # -------- expansion pass B: levels 0-3 + accumulate ------------
        for gi, gb in enumerate(GBORD):
            u01 = utbp.tile([128, 2, 4096], F8, tag="ut01", name=f"u01_{gb}")
            for f in range(2):
                (nc.sync if gi % 2 == 0 else nc.scalar).dma_start(
                    u01[:, f, :], ut_d[f, :, 4096 * gb : 4096 * (gb + 1)]
                )
            yo = yop.tile([B, 4096], F8, tag="yo", name=f"yo{gb}")
            for gg in range(8):
                g = 8 * gb + gg
                eps_t = erng[(8 * gi + gg) % 7].tile(
                    [64, 512], F32, tag="t", name=f"epsB{g}"
                )
                eps = eps_t[:, 0:512]
                sl = slice(512 * gg, 512 * (gg + 1))
                nc.tensor.matmul(
                    eps, P01[:, g // 32, :, :], u01[:, :, sl],
                    start=True, stop=True, perf_mode=DR,
                )
                next(dr_eng).tensor_tensor(
                    yo[:, sl], eps, y23s[gb][:, sl], op=ADD
                )
            (nc.sync if gi % 2 == 0 else nc.scalar).dma_start(
                corr_d[:, 4096 * gb : 4096 * (gb + 1)], yo[:]
            )

    nc.compile()
    return nc


def _pack_inputs(x, diag, u):
    """Build per-core input maps. x (B,N,1) f32, u (DEPTH,N,R) f32."""
    in_maps = []
    x2 = np.asarray(x).reshape(B, N)
    u3 = np.asarray(u)
    idn = np.eye(64, dtype=BF16)
    for c in range(NCORES):
        base = c * M
        xsl = x2[:, base : base + M]                      # (B, M)
        us = (u3[:, base : base + M, :] * USCALE).astype(np.float32)
        xt = np.ascontiguousarray(
            xsl.T.reshape(CH, 128, B).transpose(1, 0, 2)
        ).astype(FP8)                                     # [128, CH, B]
        unm = us.transpose(1, 0, 2).reshape(M, 512)       # [n, l*64+r]
        up4 = unm.reshape(CP, 2, 128, 512).transpose(2, 0, 1, 3)
        upa = np.ascontiguousarray(up4[..., 0:192]).astype(FP8)
        upb = np.ascontiguousarray(up4[..., 192:512]).astype(FP8)
        ut = np.ascontiguousarray(
            us.transpose(0, 2, 1).reshape(512, M).reshape(4, 128, M)
        ).astype(FP8)
        msk = np.zeros((64, 8, 192), dtype=BF16)
        for k in range(8):
            if (k // 4) == ((c // 4) ^ 1):
                msk[:, k, 0:64] = 1.0     # level 0
            if (k // 2) == ((c // 2) ^ 1):
                msk[:, k, 64:128] = 1.0   # level 1
            if k == c ^ 1:
                msk[:, k, 128:192] = 1.0  # level 2
        in_maps.append(
            {"xt": xt, "upa": upa, "upb": upb, "ut": ut, "mask": msk,
             "idn": idn}
        )
    return in_maps


last_results = None


def kernel(x, diag, u):
    global last_results
    from concourse.bass_utils import run_bass_kernel_spmd

    if "nc" not in _cached:
        _cached["nc"] = _build_bass()
    nc = _cached["nc"]

    in_maps = _pack_inputs(x, diag, u)
    res = run_bass_kernel_spmd(nc, in_maps, core_ids=list(range(NCORES)))
    last_results = res

    x2 = np.asarray(x, dtype=np.float32).reshape(B, N)
    d2 = np.asarray(diag, dtype=np.float32).reshape(1, N)
    y = d2 * x2
    inv = 1.0 / (USCALE * USCALE)
    for c in range(NCORES):
        y[:, c * M : (c + 1) * M] += res.results[c]["corr"].astype(np.float32) * inv
    return y.reshape(B, N, 1).astype(np.float32)
